# revision 20
# baseline (speedup 1.0000x reference)
"""Multi-head attention on 8 Trainium2 NeuronCores.

Problem: B=2, T=2048, D=1024, H=16 heads (dh=64), int 0/1 attention mask.

Sharding (hardcoded): core c -> batch b = c//4, head block hb = c%4
(4 heads = 256 cols per core). Wq/Wk/Wv column-sharded, Wo row-sharded;
each core returns a partial [T, D] output, host sums the 4 partials per
batch and adds bo.

Per-core kernel (all matmul inputs fp16, fp32 accumulation):
  phase 1: Q^T = (Wq_c)^T X^T (scaled+bias via ACT), K^T likewise,
           V = X Wv_c + bv (bias via K=1 ones matmul), V augmented with a
           ones column per head (denominator trick).
  phase 2 (per head, per 128-row k-tile of the T axis):
           S^T[k,q] = K_h Q_h^T  ->  E = exp(S^T)  ->  E *= mask^T tile
           U_aug^T += V_aug_h[k-tile]^T E            (PSUM accum over k)
           V_aug has the ones column replicated 64x, so U_aug^T rows
           64:128 all hold the softmax denominator row -- the matmul
           itself broadcasts it; normalize = reciprocal + one multiply.
  phase 3: O_partial = Hcat^T.T Wo_c  -> DMA out as fp16 (summed in f32
           on the host).

No max-subtraction is needed: scores are O(1) (exp range ~e^-6..e^6) and
softmax(x) == softmax(x - max) exactly in the masked-multiplicative form
E = exp(S) * m / sum(exp(S) * m).
"""
import contextlib
import os
import sys
import time

# more robust against a previously wedged device; must be set before the
# jax/axon backend initializes
os.environ.setdefault("NEURON_RT_RESET_CORES", "1")

if "/opt/trn_rl_repo" not in sys.path:
    sys.path.insert(0, "/opt/trn_rl_repo")

import numpy as np

import concourse.bass as bass  # noqa: F401  (import keeps bass registered)
from concourse import bacc
import concourse.mybir as mybir
import concourse.tile as tile
from concourse.bass_utils import run_bass_kernel_spmd

f32 = mybir.dt.float32
f16 = mybir.dt.float16
AF = mybir.ActivationFunctionType

B, T, D, H = 2, 2048, 1024, 16
DH = 64                 # head dim
NHC = 4                 # heads per core
C = NHC * DH            # 256 columns per core
KD = D // 128           # 8 contraction tiles over D
KT = T // 128           # 16 k-tiles over T
QC = T // 512           # 4 q chunks of 512
NCORES = 8
SCALE = DH ** -0.5      # 0.125

_CACHE = {}


def _build(repeat=1):
    nc = bacc.Bacc()
    xt = nc.declare_dram_parameter("xt", [D, T], f16, isOutput=False)
    wq = nc.declare_dram_parameter("wq", [D, C], f16, isOutput=False)
    wk = nc.declare_dram_parameter("wk", [D, C], f16, isOutput=False)
    wv = nc.declare_dram_parameter("wv", [D, C], f16, isOutput=False)
    wo = nc.declare_dram_parameter("wo", [C, D], f16, isOutput=False)
    maskt = nc.declare_dram_parameter("maskt", [T, T], f16, isOutput=False)
    bqs = nc.declare_dram_parameter("bqs", [C], f32, isOutput=False)
    bks = nc.declare_dram_parameter("bks", [C], f32, isOutput=False)
    bvr = nc.declare_dram_parameter("bvr", [1, C], f16, isOutput=False)
    out = nc.declare_dram_parameter("out", [T, D], f16, isOutput=True)

    with tile.TileContext(nc) as tc:
        loop_ctx = tc.For_i(0, repeat, 1) if repeat > 1 else contextlib.nullcontext()
        with (
            loop_ctx,
            tc.tile_pool(name="persist", bufs=1) as pp,
            tc.tile_pool(name="e", bufs=5) as ep,
            tc.tile_pool(name="osb", bufs=6) as op_,
            tc.tile_pool(name="small", bufs=1) as sp,
        ):
            xt_sb = pp.tile([128, KD, T], f16)
            wq_sb = pp.tile([128, KD, C], f16)
            wk_sb = pp.tile([128, KD, C], f16)
            wv_sb = pp.tile([128, KD, C], f16)
            wo_sb = pp.tile([128, C // 128, D], f16)
            mk_sb = pp.tile([128, KT, T], f16)
            qt_sb = pp.tile([128, C // 128, T], f16)
            kt_sb = pp.tile([128, C // 128, T], f16)
            v_sb = pp.tile([128, KT, NHC * 2 * DH], f16)
            hc_sb = pp.tile([128, C // 128, T], f16)
            bq_sb = pp.tile([128, C // 128], f32)
            bk_sb = pp.tile([128, C // 128], f32)
            bv_sb = pp.tile([1, C], f16)
            ones128 = pp.tile([1, 128], f16)

            # ---- input DMAs ----
            # Weights + xt interleaved per k-tile on HWDGE so phase-1 matmuls
            # start as soon as the first tiles land; mask tiles (phase 2
            # only) go on the SWDGE queues via gpsimd.
            xt_r = xt.rearrange("(kd p) t -> p kd t", p=128)
            wq_r = wq.rearrange("(kd p) c -> p kd c", p=128)
            wk_r = wk.rearrange("(kd p) c -> p kd c", p=128)
            wv_r = wv.rearrange("(kd p) c -> p kd c", p=128)
            # startup-critical order: wq, then qc0's xt in 2-ktile pieces so
            # the first projection group can chase the DMA wave
            nc.sync.dma_start(out=wq_sb[:, 0:2, :], in_=wq_r[:, 0:2, :])
            nc.sync.dma_start(out=wq_sb[:, 2:KD, :], in_=wq_r[:, 2:KD, :])
            for kd2 in range(0, KD, 2):
                nc.sync.dma_start(
                    out=xt_sb[:, kd2 : kd2 + 2, 0:512],
                    in_=xt_r[:, kd2 : kd2 + 2, 0:512],
                )
            nc.sync.dma_start(out=bq_sb, in_=bqs.rearrange("(m p) -> p m", p=128))
            nc.sync.dma_start(out=wk_sb, in_=wk_r)
            nc.sync.dma_start(out=bk_sb, in_=bks.rearrange("(m p) -> p m", p=128))
            nc.sync.dma_start(out=wv_sb, in_=wv_r)
            nc.sync.dma_start(out=bv_sb, in_=bvr[:, :])
            for qc in range(1, QC):
                nc.sync.dma_start(
                    out=xt_sb[:, :, qc * 512 : (qc + 1) * 512],
                    in_=xt_r[:, :, qc * 512 : (qc + 1) * 512],
                )
            nc.sync.dma_start(out=wo_sb, in_=wo.rearrange("(m p) d -> p m d", p=128))
            nc.vector.memset(ones128, 1.0)
            v4 = v_sb.rearrange("p kt (h x) -> p kt h x", x=2 * DH)
            nc.vector.memset(v4[:, :, :, DH:], 1.0)

            # ---- phase 1: projections ----
            with tc.tile_pool(name="ps1", bufs=2, space="PSUM") as ps1:
                for qc in range(QC):
                    for w_sb, b_sb, dst, scale in (
                        (wq_sb, bq_sb, qt_sb, SCALE),
                        (wk_sb, bk_sb, kt_sb, 1.0),
                    ):
                        for m in range(C // 128):
                            pt = ps1.tile([128, 512], f32, tag="p")
                            for kd in range(KD):
                                nc.tensor.matmul(
                                    pt,
                                    w_sb[:, kd, m * 128 : (m + 1) * 128],
                                    xt_sb[:, kd, qc * 512 : (qc + 1) * 512],
                                    start=(kd == 0),
                                    stop=(kd == KD - 1),
                                )
                            nc.scalar.activation(
                                dst[:, m, qc * 512 : (qc + 1) * 512],
                                pt,
                                AF.Identity,
                                bias=b_sb[:, m : m + 1],
                                scale=scale,
                            )
                    for tt in range(4):
                        t = qc * 4 + tt
                        pv = ps1.tile([128, C], f32, tag="v")
                        for kd in range(KD):
                            nc.tensor.matmul(
                                pv,
                                xt_sb[:, kd, t * 128 : (t + 1) * 128],
                                wv_sb[:, kd, :],
                                start=(kd == 0),
                                stop=False,
                            )
                        nc.tensor.matmul(pv, ones128, bv_sb, start=False, stop=True)
                        nc.vector.tensor_copy(
                            v4[:, t, :, 0:DH],
                            pv.rearrange("p (h x) -> p h x", x=DH),
                        )

            # mask tiles are only needed in phase 2; emitting their DMAs
            # after phase 1 keeps the startup HWDGE/bandwidth free for xt+w
            mk_r = maskt.rearrange("(kt p) t -> p kt t", p=128)
            for kt in range(KT):
                nc.gpsimd.dma_start(out=mk_sb[:, kt, :], in_=mk_r[:, kt, :])

            # ---- phase 2: attention per head ----
            with (
                tc.tile_pool(name="ps_s", bufs=2, space="PSUM") as pss,
                tc.tile_pool(name="ps_u", bufs=1, space="PSUM") as psu,
            ):
                def s_matmuls(h, kt):
                    m, p0 = h // 2, (h % 2) * 64
                    halves = []
                    ctx2 = tc.high_priority(offset=24)
                    ctx2.__enter__()
                    for half in range(2):
                        st = pss.tile([128, 1024], f32, tag="s")
                        for sub in range(2):
                            qc = half * 2 + sub
                            nc.tensor.matmul(
                                st[:, sub * 512 : (sub + 1) * 512],
                                kt_sb[p0 : p0 + 64, m, kt * 128 : (kt + 1) * 128],
                                qt_sb[p0 : p0 + 64, m, qc * 512 : (qc + 1) * 512],
                                start=True,
                                stop=True,
                            )
                        halves.append(st)
                    ctx2.__exit__(None, None, None)
                    return halves

                # software pipeline: S matmuls for step i+1 are emitted on PE
                # before the (DVE-gated) U matmuls of step i, so ACT's exp
                # stream never waits behind PE head-of-line blocking.
                steps = [(h, kt) for h in range(NHC) for kt in range(KT)]
                st_next = s_matmuls(*steps[0])
                u = None
                for i, (h, kt) in enumerate(steps):
                    m, p0 = h // 2, (h % 2) * 64
                    if kt == 0:
                        u = psu.tile([2 * DH, T], f32, tag="u")
                    st_cur = st_next
                    if i + 1 < len(steps):
                        st_next = s_matmuls(*steps[i + 1])
                    e = ep.tile([128, T], f16)
                    for half in range(2):
                        nc.scalar.activation(
                            e[:, half * 1024 : (half + 1) * 1024],
                            st_cur[half],
                            AF.Exp,
                        )
                    nc.vector.tensor_mul(e, e, mk_sb[:, kt, :])
                    for qc in range(QC):
                        nc.tensor.matmul(
                            u[:, qc * 512 : (qc + 1) * 512],
                            v_sb[:, kt, h * 2 * DH : (h + 1) * 2 * DH],
                            e[:, qc * 512 : (qc + 1) * 512],
                            start=(kt == 0),
                            stop=(kt == KT - 1),
                        )
                    if kt == KT - 1:
                        with tc.high_priority(offset=40):
                            recb = sp.tile([64, T], f32, tag="recb")
                            for half in range(2):
                                sl = slice(half * 1024, (half + 1) * 1024)
                                nc.vector.reciprocal(recb[:, sl], u[DH : 2 * DH, sl])
                                nc.vector.tensor_mul(
                                    hc_sb[p0 : p0 + 64, m, sl],
                                    u[0:DH, sl],
                                    recb[:, sl],
                                )

            # ---- phase 3: output projection ----
            with tc.tile_pool(name="ps_o", bufs=6, space="PSUM") as pso:
                for t in range(KT):
                    ot = op_.tile([128, 1024], f16)
                    for n in range(2):
                        po = pso.tile([128, 512], f32, tag="o")
                        for m in range(C // 128):
                            nc.tensor.matmul(
                                po,
                                hc_sb[:, m, t * 128 : (t + 1) * 128],
                                wo_sb[:, m, n * 512 : (n + 1) * 512],
                                start=(m == 0),
                                stop=(m == C // 128 - 1),
                            )
                        if (t * 2 + n) % 2 == 0:
                            nc.vector.tensor_copy(ot[:, n * 512 : (n + 1) * 512], po)
                        else:
                            nc.scalar.activation(
                                ot[:, n * 512 : (n + 1) * 512], po, AF.Identity
                            )
                    nc.sync.dma_start(
                        out=out[t * 128 : (t + 1) * 128, :],
                        in_=ot,
                    )
    nc.compile()
    return nc


def _get_nc(repeat=1):
    key = ("nc", repeat)
    if key not in _CACHE:
        _CACHE[key] = _build(repeat)
    return _CACHE[key]


def _prep_core_inputs(c, x, mask, Wq, bq, Wk, bk, Wv, bv, Wo):
    b, hb = divmod(c, NCORES // B)
    sl = slice(hb * C, (hb + 1) * C)
    return {
        "xt": np.ascontiguousarray(x[b].T).astype(np.float16),
        "wq": np.ascontiguousarray(Wq[:, sl]).astype(np.float16),
        "wk": np.ascontiguousarray(Wk[:, sl]).astype(np.float16),
        "wv": np.ascontiguousarray(Wv[:, sl]).astype(np.float16),
        "wo": np.ascontiguousarray(Wo[sl, :]).astype(np.float16),
        "maskt": np.ascontiguousarray(mask[b].T).astype(np.float16),
        "bqs": (bq[sl] * SCALE).astype(np.float32),
        "bks": bk[sl].astype(np.float32),
        "bvr": bv[sl].astype(np.float16).reshape(1, C),
    }


def kernel(
    inputs, mask, Wq, bq, Wk, bk, Wv, bv, Wo, bo,
    _trace=False, _trace_kwargs=None, _repeat=1,
):
    x = np.asarray(inputs, dtype=np.float32)
    mask = np.asarray(mask)
    Wq, bq = np.asarray(Wq, np.float32), np.asarray(bq, np.float32)
    Wk, bk = np.asarray(Wk, np.float32), np.asarray(bk, np.float32)
    Wv, bv = np.asarray(Wv, np.float32), np.asarray(bv, np.float32)
    Wo, bo = np.asarray(Wo, np.float32), np.asarray(bo, np.float32)

    nc = _get_nc(_repeat)
    in_maps = [
        _prep_core_inputs(c, x, mask, Wq, bq, Wk, bk, Wv, bv, Wo)
        for c in range(NCORES)
    ]
    last_err = None
    for attempt in range(3):
        try:
            res = run_bass_kernel_spmd(
                nc,
                in_maps,
                list(range(NCORES)),
                trace=_trace,
                **(_trace_kwargs or {}),
            )
            break
        except Exception as e:  # wedged device etc. -- retry
            last_err = e
            time.sleep(3.0)
    else:
        raise last_err
    out = np.empty((B, T, D), np.float32)
    per_b = NCORES // B
    for b in range(B):
        acc = res.results[b * per_b]["out"].astype(np.float32)
        for j in range(1, per_b):
            acc = acc + res.results[b * per_b + j]["out"].astype(np.float32)
        out[b] = acc + bo[None, :]
    if _trace:
        kernel.last_results = res
    return out


# revision 21
# speedup vs baseline: 1.0072x; 1.0072x over previous
"""Multi-head attention on 8 Trainium2 NeuronCores.

Problem: B=2, T=2048, D=1024, H=16 heads (dh=64), int 0/1 attention mask.

Sharding (hardcoded): core c -> batch b = c//4, head block hb = c%4
(4 heads = 256 cols per core). Wq/Wk/Wv column-sharded, Wo row-sharded;
each core returns a partial [T, D] output, host sums the 4 partials per
batch and adds bo.

Per-core kernel (all matmul inputs fp16, fp32 accumulation):
  phase 1: Q^T = (Wq_c)^T X^T (scaled+bias via ACT), K^T likewise,
           V = X Wv_c + bv (bias via K=1 ones matmul), V augmented with a
           ones column per head (denominator trick).
  phase 2 (per head, per 128-row k-tile of the T axis):
           S^T[k,q] = K_h Q_h^T  ->  E = exp(S^T)  ->  E *= mask^T tile
           U_aug^T += V_aug_h[k-tile]^T E            (PSUM accum over k)
           V_aug has the ones column replicated 64x, so U_aug^T rows
           64:128 all hold the softmax denominator row -- the matmul
           itself broadcasts it; normalize = reciprocal + one multiply.
  phase 3: O_partial = Hcat^T.T Wo_c  -> DMA out as fp16 (summed in f32
           on the host).

No max-subtraction is needed: scores are O(1) (exp range ~e^-6..e^6) and
softmax(x) == softmax(x - max) exactly in the masked-multiplicative form
E = exp(S) * m / sum(exp(S) * m).
"""
import contextlib
import os
import sys
import time

# more robust against a previously wedged device; must be set before the
# jax/axon backend initializes
os.environ.setdefault("NEURON_RT_RESET_CORES", "1")

if "/opt/trn_rl_repo" not in sys.path:
    sys.path.insert(0, "/opt/trn_rl_repo")

import numpy as np

import concourse.bass as bass  # noqa: F401  (import keeps bass registered)
from concourse import bacc
import concourse.mybir as mybir
import concourse.tile as tile
from concourse.bass_utils import run_bass_kernel_spmd

f32 = mybir.dt.float32
f16 = mybir.dt.float16
AF = mybir.ActivationFunctionType

B, T, D, H = 2, 2048, 1024, 16
DH = 64                 # head dim
NHC = 4                 # heads per core
C = NHC * DH            # 256 columns per core
KD = D // 128           # 8 contraction tiles over D
KT = T // 128           # 16 k-tiles over T
QC = T // 512           # 4 q chunks of 512
NCORES = 8
SCALE = DH ** -0.5      # 0.125

_CACHE = {}


def _build(repeat=1):
    nc = bacc.Bacc()
    xt = nc.declare_dram_parameter("xt", [D, T], f16, isOutput=False)
    wq = nc.declare_dram_parameter("wq", [D, C], f16, isOutput=False)
    wk = nc.declare_dram_parameter("wk", [D, C], f16, isOutput=False)
    wv = nc.declare_dram_parameter("wv", [D, C], f16, isOutput=False)
    wo = nc.declare_dram_parameter("wo", [C, D], f16, isOutput=False)
    maskt = nc.declare_dram_parameter("maskt", [T, T], f16, isOutput=False)
    bqs = nc.declare_dram_parameter("bqs", [C], f32, isOutput=False)
    bks = nc.declare_dram_parameter("bks", [C], f32, isOutput=False)
    bvr = nc.declare_dram_parameter("bvr", [1, C], f16, isOutput=False)
    out = nc.declare_dram_parameter("out", [T, D], f16, isOutput=True)

    with tile.TileContext(nc) as tc:
        loop_ctx = tc.For_i(0, repeat, 1) if repeat > 1 else contextlib.nullcontext()
        with (
            loop_ctx,
            tc.tile_pool(name="persist", bufs=1) as pp,
            tc.tile_pool(name="e", bufs=7) as ep,
            tc.tile_pool(name="osb", bufs=6) as op_,
            tc.tile_pool(name="small", bufs=1) as sp,
        ):
            xt_sb = pp.tile([128, KD, T], f16)
            wq_sb = pp.tile([128, KD, C], f16)
            wk_sb = pp.tile([128, KD, C], f16)
            wv_sb = pp.tile([128, KD, C], f16)
            wo_sb = pp.tile([128, C // 128, D], f16)
            mk_sb = pp.tile([128, KT, T], f16)
            qt_sb = pp.tile([128, C // 128, T], f16)
            kt_sb = pp.tile([128, C // 128, T], f16)
            v_sb = pp.tile([128, KT, NHC * 2 * DH], f16)
            hc_sb = pp.tile([128, C // 128, T], f16)
            bq_sb = pp.tile([128, C // 128], f32)
            bk_sb = pp.tile([128, C // 128], f32)
            bv_sb = pp.tile([1, C], f16)
            ones128 = pp.tile([1, 128], f16)

            # ---- input DMAs ----
            # Weights + xt interleaved per k-tile on HWDGE so phase-1 matmuls
            # start as soon as the first tiles land; mask tiles (phase 2
            # only) go on the SWDGE queues via gpsimd.
            xt_r = xt.rearrange("(kd p) t -> p kd t", p=128)
            wq_r = wq.rearrange("(kd p) c -> p kd c", p=128)
            wk_r = wk.rearrange("(kd p) c -> p kd c", p=128)
            wv_r = wv.rearrange("(kd p) c -> p kd c", p=128)
            # startup-critical order: wq, then qc0's xt in 2-ktile pieces so
            # the first projection group can chase the DMA wave
            nc.sync.dma_start(out=wq_sb[:, 0:2, :], in_=wq_r[:, 0:2, :])
            nc.sync.dma_start(out=wq_sb[:, 2:KD, :], in_=wq_r[:, 2:KD, :])
            for kd2 in range(0, KD, 2):
                nc.sync.dma_start(
                    out=xt_sb[:, kd2 : kd2 + 2, 0:512],
                    in_=xt_r[:, kd2 : kd2 + 2, 0:512],
                )
            nc.sync.dma_start(out=bq_sb, in_=bqs.rearrange("(m p) -> p m", p=128))
            nc.sync.dma_start(out=wk_sb, in_=wk_r)
            nc.sync.dma_start(out=bk_sb, in_=bks.rearrange("(m p) -> p m", p=128))
            nc.sync.dma_start(out=wv_sb, in_=wv_r)
            nc.sync.dma_start(out=bv_sb, in_=bvr[:, :])
            for qc in range(1, QC):
                nc.sync.dma_start(
                    out=xt_sb[:, :, qc * 512 : (qc + 1) * 512],
                    in_=xt_r[:, :, qc * 512 : (qc + 1) * 512],
                )
            nc.sync.dma_start(out=wo_sb, in_=wo.rearrange("(m p) d -> p m d", p=128))
            nc.vector.memset(ones128, 1.0)
            v4 = v_sb.rearrange("p kt (h x) -> p kt h x", x=2 * DH)
            nc.vector.memset(v4[:, :, :, DH:], 1.0)

            # ---- phase 1: projections ----
            with tc.tile_pool(name="ps1", bufs=2, space="PSUM") as ps1:
                for qc in range(QC):
                    for w_sb, b_sb, dst, scale in (
                        (wq_sb, bq_sb, qt_sb, SCALE),
                        (wk_sb, bk_sb, kt_sb, 1.0),
                    ):
                        for m in range(C // 128):
                            pt = ps1.tile([128, 512], f32, tag="p")
                            for kd in range(KD):
                                nc.tensor.matmul(
                                    pt,
                                    w_sb[:, kd, m * 128 : (m + 1) * 128],
                                    xt_sb[:, kd, qc * 512 : (qc + 1) * 512],
                                    start=(kd == 0),
                                    stop=(kd == KD - 1),
                                )
                            nc.scalar.activation(
                                dst[:, m, qc * 512 : (qc + 1) * 512],
                                pt,
                                AF.Identity,
                                bias=b_sb[:, m : m + 1],
                                scale=scale,
                            )
                    for tt in range(4):
                        t = qc * 4 + tt
                        pv = ps1.tile([128, C], f32, tag="v")
                        for kd in range(KD):
                            nc.tensor.matmul(
                                pv,
                                xt_sb[:, kd, t * 128 : (t + 1) * 128],
                                wv_sb[:, kd, :],
                                start=(kd == 0),
                                stop=False,
                            )
                        nc.tensor.matmul(pv, ones128, bv_sb, start=False, stop=True)
                        nc.vector.tensor_copy(
                            v4[:, t, :, 0:DH],
                            pv.rearrange("p (h x) -> p h x", x=DH),
                        )

            # mask tiles are only needed in phase 2; emitting their DMAs
            # after phase 1 keeps the startup HWDGE/bandwidth free for xt+w
            mk_r = maskt.rearrange("(kt p) t -> p kt t", p=128)
            for kt in range(KT):
                nc.gpsimd.dma_start(out=mk_sb[:, kt, :], in_=mk_r[:, kt, :])

            # ---- phase 2: attention per head ----
            with (
                tc.tile_pool(name="ps_s", bufs=2, space="PSUM") as pss,
                tc.tile_pool(name="ps_u", bufs=1, space="PSUM") as psu,
            ):
                def s_matmuls(h, kt):
                    m, p0 = h // 2, (h % 2) * 64
                    halves = []
                    ctx2 = tc.high_priority(offset=24)
                    ctx2.__enter__()
                    for half in range(2):
                        st = pss.tile([128, 1024], f32, tag="s")
                        for sub in range(2):
                            qc = half * 2 + sub
                            nc.tensor.matmul(
                                st[:, sub * 512 : (sub + 1) * 512],
                                kt_sb[p0 : p0 + 64, m, kt * 128 : (kt + 1) * 128],
                                qt_sb[p0 : p0 + 64, m, qc * 512 : (qc + 1) * 512],
                                start=True,
                                stop=True,
                            )
                        halves.append(st)
                    ctx2.__exit__(None, None, None)
                    return halves

                # software pipeline: S matmuls for step i+1 are emitted on PE
                # before the (DVE-gated) U matmuls of step i, so ACT's exp
                # stream never waits behind PE head-of-line blocking.
                steps = [(h, kt) for h in range(NHC) for kt in range(KT)]
                st_next = s_matmuls(*steps[0])
                u = None
                for i, (h, kt) in enumerate(steps):
                    m, p0 = h // 2, (h % 2) * 64
                    if kt == 0:
                        u = psu.tile([2 * DH, T], f32, tag="u")
                    st_cur = st_next
                    if i + 1 < len(steps):
                        st_next = s_matmuls(*steps[i + 1])
                    e = ep.tile([128, T], f16)
                    for half in range(2):
                        nc.scalar.activation(
                            e[:, half * 1024 : (half + 1) * 1024],
                            st_cur[half],
                            AF.Exp,
                        )
                    nc.vector.tensor_mul(e, e, mk_sb[:, kt, :])
                    for qc in range(QC):
                        nc.tensor.matmul(
                            u[:, qc * 512 : (qc + 1) * 512],
                            v_sb[:, kt, h * 2 * DH : (h + 1) * 2 * DH],
                            e[:, qc * 512 : (qc + 1) * 512],
                            start=(kt == 0),
                            stop=(kt == KT - 1),
                        )
                    if kt == KT - 1:
                        with tc.high_priority(offset=40):
                            recb = sp.tile([64, T], f32, tag="recb")
                            for half in range(2):
                                sl = slice(half * 1024, (half + 1) * 1024)
                                nc.vector.reciprocal(recb[:, sl], u[DH : 2 * DH, sl])
                                nc.vector.tensor_mul(
                                    hc_sb[p0 : p0 + 64, m, sl],
                                    u[0:DH, sl],
                                    recb[:, sl],
                                )

            # ---- phase 3: output projection ----
            with tc.tile_pool(name="ps_o", bufs=8, space="PSUM") as pso:
                for t in range(KT):
                    ot = op_.tile([128, 1024], f16)
                    for n in range(2):
                        po = pso.tile([128, 512], f32, tag="o")
                        for m in range(C // 128):
                            nc.tensor.matmul(
                                po,
                                hc_sb[:, m, t * 128 : (t + 1) * 128],
                                wo_sb[:, m, n * 512 : (n + 1) * 512],
                                start=(m == 0),
                                stop=(m == C // 128 - 1),
                            )
                        if (t * 2 + n) % 2 == 0:
                            nc.vector.tensor_copy(ot[:, n * 512 : (n + 1) * 512], po)
                        else:
                            nc.scalar.activation(
                                ot[:, n * 512 : (n + 1) * 512], po, AF.Identity
                            )
                    nc.sync.dma_start(
                        out=out[t * 128 : (t + 1) * 128, :],
                        in_=ot,
                    )
    nc.compile()
    return nc


def _get_nc(repeat=1):
    key = ("nc", repeat)
    if key not in _CACHE:
        _CACHE[key] = _build(repeat)
    return _CACHE[key]


def _prep_core_inputs(c, x, mask, Wq, bq, Wk, bk, Wv, bv, Wo):
    b, hb = divmod(c, NCORES // B)
    sl = slice(hb * C, (hb + 1) * C)
    return {
        "xt": np.ascontiguousarray(x[b].T).astype(np.float16),
        "wq": np.ascontiguousarray(Wq[:, sl]).astype(np.float16),
        "wk": np.ascontiguousarray(Wk[:, sl]).astype(np.float16),
        "wv": np.ascontiguousarray(Wv[:, sl]).astype(np.float16),
        "wo": np.ascontiguousarray(Wo[sl, :]).astype(np.float16),
        "maskt": np.ascontiguousarray(mask[b].T).astype(np.float16),
        "bqs": (bq[sl] * SCALE).astype(np.float32),
        "bks": bk[sl].astype(np.float32),
        "bvr": bv[sl].astype(np.float16).reshape(1, C),
    }


def kernel(
    inputs, mask, Wq, bq, Wk, bk, Wv, bv, Wo, bo,
    _trace=False, _trace_kwargs=None, _repeat=1,
):
    x = np.asarray(inputs, dtype=np.float32)
    mask = np.asarray(mask)
    Wq, bq = np.asarray(Wq, np.float32), np.asarray(bq, np.float32)
    Wk, bk = np.asarray(Wk, np.float32), np.asarray(bk, np.float32)
    Wv, bv = np.asarray(Wv, np.float32), np.asarray(bv, np.float32)
    Wo, bo = np.asarray(Wo, np.float32), np.asarray(bo, np.float32)

    nc = _get_nc(_repeat)
    in_maps = [
        _prep_core_inputs(c, x, mask, Wq, bq, Wk, bk, Wv, bv, Wo)
        for c in range(NCORES)
    ]
    last_err = None
    for attempt in range(3):
        try:
            res = run_bass_kernel_spmd(
                nc,
                in_maps,
                list(range(NCORES)),
                trace=_trace,
                **(_trace_kwargs or {}),
            )
            break
        except Exception as e:  # wedged device etc. -- retry
            last_err = e
            time.sleep(3.0)
    else:
        raise last_err
    out = np.empty((B, T, D), np.float32)
    per_b = NCORES // B
    for b in range(B):
        acc = res.results[b * per_b]["out"].astype(np.float32)
        for j in range(1, per_b):
            acc = acc + res.results[b * per_b + j]["out"].astype(np.float32)
        out[b] = acc + bo[None, :]
    if _trace:
        kernel.last_results = res
    return out
